# revision 7
# baseline (speedup 1.0000x reference)
"""Multi-head attention (RoPE + mask + softmax) Trainium2 Bass kernel, v2.

Sharding: 8 cores = 2 batches x 4 head-groups. Core c handles batch c//4,
local heads 4*(c%4) .. +4 (tensor-parallel on heads; Wq/Wk/Wv column-sharded,
Wo row-sharded; per-core partial outputs summed on host).

v2 design (from v1 trace: PE cold-throttled 82% of span, 129us of PE gaps,
ACT exp = 142us floor):
  - everything on the matmul path is bf16 (half the DMA, full PE rate, FWL)
  - scores computed as ROW-PACKED concurrent pairs: head h2=0 on PE rows
    0-63, h2=1 on rows 64-127 (tile_position auto-derived from base
    partitions) -> both heads' scores in one 512-cycle stream
  - projections are emission-interleaved into the first attention unit's
    m-loop so exp starts early and the PE never idles (HAM stays warm)
  - softmax denominators ride as a 65th "ones" column of vh; the reciprocal/
    normalize chain never touches PE or ACT: psum rows DMA to a DRAM bounce,
    DVE reciprocal on a [128,2,4] transpose, broadcast-DMA back to SBUF,
    one DVE multiply -> outTb
  - avp psum banks are freed by DMA eviction immediately at unit end
  - out-projection is head-pair packed (K=128, 2 accumulation steps) and its
    psum is DMA'd straight to DRAM
  - PSUM: shared pool 3x[128,2,512] (6 banks) + 2x avp [128,512] (2 banks)
"""
import sys
sys.path.insert(0, '/opt/trn_rl_repo')
import math
import numpy as np
import ml_dtypes

import concourse.bass as bass
import concourse.mybir as mybir
import concourse.tile as tile
from concourse import bacc
from concourse.bass_utils import run_bass_kernel_spmd

F32 = mybir.dt.float32
BF16 = mybir.dt.bfloat16

S = 2048
DIM = 1024
HEAD_DIM = 64
N_CORES = 8
KC = DIM // 128          # 8 contraction chunks for projections
MT = S // 128            # 16 k-chunks in attention
QB = 512                 # q-block width
NQB = S // QB            # 4
ROPE_THETA = 10000.0

_BUILT = None


def build_bass():
    nc = bacc.Bacc("TRN2", target_bir_lowering=False, debug=False)

    xq = nc.dram_tensor("xq", [4, 128, KC, 512], BF16, kind="ExternalInput").ap()
    xk = nc.dram_tensor("xk", [4, 128, KC, 512], BF16, kind="ExternalInput").ap()
    xv = nc.dram_tensor("xv", [MT, 128, KC, 128], BF16, kind="ExternalInput").ap()
    wq = nc.dram_tensor("wq", [128, KC, 256], BF16, kind="ExternalInput").ap()
    wk = nc.dram_tensor("wk", [128, KC, 256], BF16, kind="ExternalInput").ap()
    wv = nc.dram_tensor("wv", [128, KC, 256], BF16, kind="ExternalInput").ap()
    wo = nc.dram_tensor("wo", [128, 2, DIM], BF16, kind="ExternalInput").ap()
    cosT = nc.dram_tensor("cosT", [128, S], F32, kind="ExternalInput").ap()
    sinT = nc.dram_tensor("sinT", [128, S], F32, kind="ExternalInput").ap()
    maskT = nc.dram_tensor("maskT", [128, MT, S], BF16, kind="ExternalInput").ap()
    out_part = nc.dram_tensor("out_part", [S, DIM], F32, kind="ExternalOutput").ap()

    with tile.TileContext(nc) as tc:
        with tc.tile_pool(name="persist", bufs=1) as persist, \
             tc.tile_pool(name="dram", bufs=1, space="DRAM") as dram, \
             tc.tile_pool(name="ps", bufs=3, space="PSUM") as ps, \
             tc.tile_pool(name="avpp", bufs=2, space="PSUM") as avpp, \
             tc.tile_pool(name="xts", bufs=2) as xts, \
             tc.tile_pool(name="rope", bufs=2) as rope, \
             tc.tile_pool(name="attn", bufs=3) as attnp, \
             tc.tile_pool(name="tailp", bufs=2) as tailp:

            qhT = persist.tile([128, 2, S], BF16)     # [64*h2-chunk, hp, s]
            khT = persist.tile([128, 2, S], BF16)
            vh = persist.tile([128, MT, 4 * 65], BF16)
            outTb = persist.tile([128, 2, S], BF16)   # [pair-packed hd, hp, s]
            wo_sb = persist.tile([128, 2, DIM], BF16)
            cos_sb = persist.tile([128, S], F32)
            sin_sb = persist.tile([128, S], F32)
            mk = persist.tile([128, MT, S], BF16)
            wq_sb = persist.tile([128, KC, 256], BF16)
            wk_sb = persist.tile([128, KC, 256], BF16)
            wv_sb = persist.tile([128, KC, 256], BF16)
            dscr = dram.tile([16, QB], F32)
            dscr2 = dram.tile([16, QB], F32)

            nc.sync.dma_start(out=wq_sb, in_=wq)
            nc.sync.dma_start(out=wk_sb, in_=wk)
            nc.sync.dma_start(out=wv_sb, in_=wv)
            nc.sync.dma_start(out=wo_sb, in_=wo)
            nc.sync.dma_start(out=cos_sb, in_=cosT)
            nc.sync.dma_start(out=sin_sb, in_=sinT)
            # ones column per head for the softmax denominator rows of vh
            nc.vector.memset(
                vh.rearrange("p m (h x) -> p m h x", x=65)[:, :, :, 64:65], 1.0)

            # ---------- emission helpers ----------

            def emit_qk_sblk(xdram, w_sb, dstT, sblk):
                """Project one 512-token block of q or k and apply RoPE."""
                x_sb = xts.tile([128, KC, 512], BF16, tag="xts")
                nc.sync.dma_start(out=x_sb, in_=xdram[sblk])
                ss = slice(sblk * 512, (sblk + 1) * 512)
                psum = ps.tile([128, 2, 512], F32, tag="ps")
                for m in range(2):
                    for kc in range(KC):
                        nc.tensor.matmul(
                            psum[:, m, :],
                            lhsT=w_sb[:, kc, m * 128:(m + 1) * 128],
                            rhs=x_sb[:, kc, :],
                            start=(kc == 0), stop=(kc == KC - 1))
                t = rope.tile([128, 2, 512], F32, tag="t")
                u = rope.tile([128, 2, 512], F32, tag="u")
                us = rope.tile([128, 2, 512], F32, tag="us")
                cosb = cos_sb[:, ss].unsqueeze(1).broadcast_to([128, 2, 512])
                sinb = sin_sb[:, ss].unsqueeze(1).broadcast_to([128, 2, 512])
                nc.vector.tensor_mul(t, psum, cosb)
                nc.vector.tensor_mul(u, psum, sinb)
                for blk in range(4):
                    a, b2 = blk * 32, (blk ^ 1) * 32
                    nc.gpsimd.dma_start(out=us[a:a + 32], in_=u[b2:b2 + 32])
                nc.vector.tensor_add(dstT[:, :, ss], t, us)

            def emit_v_sc(sc):
                """Project one 128-token chunk of v into vh (65-stride)."""
                v_sb = xts.tile([128, KC, 128], BF16, tag="vts")
                nc.sync.dma_start(out=v_sb, in_=xv[sc])
                psum = ps.tile([128, 2, 512], F32, tag="ps")
                for kc in range(KC):
                    nc.tensor.matmul(
                        psum[:, 0, 0:256], lhsT=v_sb[:, kc, :],
                        rhs=wv_sb[:, kc, :],
                        start=(kc == 0), stop=(kc == KC - 1))
                nc.scalar.copy(
                    vh[:, sc, :].rearrange("p (h x) -> p h x", x=65)[:, :, 0:64],
                    psum[:, 0, 0:256].rearrange("p (h x) -> p h x", x=64))

            def emit_mask_dma(m):
                nc.sync.dma_start(out=mk[:, m, :], in_=maskT[:, m, :])

            avps = {}

            def emit_attn_iter(qb, hp, m):
                qs = slice(qb * QB, (qb + 1) * QB)
                if m == 0:
                    avps[(qb, hp)] = [
                        avpp.tile([128, QB], F32, tag="avp", name=f"avp{qb}{hp}{i}")
                        for i in range(2)]
                avp = avps[(qb, hp)]
                sps = ps.tile([128, 2, 512], F32, tag="ps")
                for h2 in range(2):
                    hb = slice(h2 * 64, (h2 + 1) * 64)
                    nc.tensor.matmul(
                        sps[:, h2, :],
                        lhsT=khT[hb, hp, m * 128:(m + 1) * 128],
                        rhs=qhT[hb, hp, qs],
                        start=True, stop=True)
                at = attnp.tile([128, 2, 512], BF16, tag="at")
                nc.scalar.activation(at, sps, mybir.ActivationFunctionType.Exp,
                                     scale=1.0 / math.sqrt(HEAD_DIM))
                atm = attnp.tile([128, 2, 512], BF16, tag="atm")
                mkb = mk[:, m, qs].unsqueeze(1).broadcast_to([128, 2, 512])
                nc.vector.tensor_mul(atm, at, mkb)
                for h2 in range(2):
                    h = 2 * hp + h2
                    nc.tensor.matmul(
                        avp[h2][0:65, :],
                        lhsT=vh[:, m, h * 65:(h + 1) * 65],
                        rhs=atm[:, h2, :],
                        start=(m == 0), stop=(m == MT - 1))

            tails = {}

            def emit_tail_evict(qb, hp):
                """Free the avp psum banks via DVE copies (DMA can't read
                PSUM); partition-shift head B into outF[64:128] by DMA."""
                u0 = (qb * 2 + hp) * 2
                avp = avps.pop((qb, hp))
                outF = tailp.tile([128, QB], F32, tag="outF")
                tmpB = tailp.tile([128, QB], F32, tag="tmpB")
                nc.vector.tensor_copy(outF[0:65, :], avp[0][0:65, :])
                nc.vector.tensor_copy(tmpB[0:65, :], avp[1][0:65, :])
                # denominator rows out first (outF[64] is then overwritten)
                nc.sync.dma_start(out=dscr[u0:u0 + 1, :], in_=outF[64:65, :])
                nc.sync.dma_start(out=dscr[u0 + 1:u0 + 2, :], in_=tmpB[64:65, :])
                nc.sync.dma_start(out=outF[64:128, :], in_=tmpB[0:64, :])
                tails[(qb, hp)] = outF

            def emit_tail_recip(qb, hp):
                u0 = (qb * 2 + hp) * 2
                rin = tailp.tile([128, 2, 4], F32, tag="rin")
                nc.sync.dma_start(
                    out=rin,
                    in_=dscr[u0:u0 + 2].rearrange("u (p f) -> p u f", p=128))
                r32 = tailp.tile([128, 2, 4], F32, tag="r32")
                scr = tailp.tile([128, 2, 4], F32, tag="scr")
                nc.vector.reciprocal_approx_accurate(r32, rin, scr)
                nc.sync.dma_start(
                    out=dscr2[u0:u0 + 2].rearrange("u (p f) -> p u f", p=128),
                    in_=r32)

            def emit_tail_norm(qb, hp):
                u0 = (qb * 2 + hp) * 2
                qs = slice(qb * QB, (qb + 1) * QB)
                outF = tails.pop((qb, hp))
                sb_r = tailp.tile([128, QB], F32, tag="sbr")
                nc.sync.dma_start(out=sb_r[0:64, :],
                                  in_=dscr2[u0:u0 + 1].broadcast_to([64, QB]))
                nc.sync.dma_start(out=sb_r[64:128, :],
                                  in_=dscr2[u0 + 1:u0 + 2].broadcast_to([64, QB]))
                nc.vector.tensor_mul(outTb[:, hp, qs], outF, sb_r)

            def emit_outproj(scs, evict_engine=None):
                ev = evict_engine or nc.vector.tensor_copy
                for sc in scs:
                    wps = ps.tile([128, 2, 512], F32, tag="ps")
                    tok = slice(sc * 128, (sc + 1) * 128)
                    for nb in range(2):
                        for hp in range(2):
                            nc.tensor.matmul(
                                wps[:, nb, :],
                                lhsT=outTb[:, hp, tok],
                                rhs=wo_sb[:, hp, nb * 512:(nb + 1) * 512],
                                start=(hp == 0), stop=(hp == 1))
                    co = tailp.tile([128, DIM], F32, tag="co")
                    ev(co, wps.rearrange("p a b -> p (a b)"))
                    nc.sync.dma_start(out=out_part[tok, :], in_=co)

            # ---------- emission schedule ----------

            emit_qk_sblk(xq, wq_sb, qhT, 0)
            emit_qk_sblk(xk, wk_sb, khT, 0)
            for sc in range(2):
                emit_v_sc(sc)
                emit_mask_dma(sc)

            # unit (qb=0, hp=0) interleaved with the rest of the projections
            emit_attn_iter(0, 0, 0)
            for sc in (2, 3):
                emit_v_sc(sc)
                emit_mask_dma(sc)
            emit_attn_iter(0, 0, 1)
            emit_attn_iter(0, 0, 2)
            emit_attn_iter(0, 0, 3)
            for sblk in range(1, 4):
                emit_qk_sblk(xk, wk_sb, khT, sblk)
                emit_qk_sblk(xq, wq_sb, qhT, sblk)
                for i in range(4):
                    sc = sblk * 4 + i
                    emit_v_sc(sc)
                    emit_mask_dma(sc)
                for i in range(4):
                    emit_attn_iter(0, 0, sblk * 4 + i)
            emit_tail_evict(0, 0)

            # remaining 7 units; tails pipelined into the following unit
            units = [(0, 1), (1, 0), (1, 1), (2, 0), (2, 1), (3, 0), (3, 1)]
            for ui, (qb, hp) in enumerate(units):
                prev = (qb, hp - 1) if hp == 1 else (qb - 1, 1)
                for m in range(MT):
                    emit_attn_iter(qb, hp, m)
                    if m == 2:
                        emit_tail_recip(*prev)
                    elif m == 5:
                        emit_tail_norm(*prev)
                    elif m == 8 and qb >= 1 and hp == 0:
                        # tokens of qb-1 are fully normalized by now
                        emit_outproj(range((qb - 1) * 4, qb * 4))
                emit_tail_evict(qb, hp)

            emit_tail_recip(3, 1)
            emit_tail_norm(3, 1)
            emit_outproj(range(12, 16), evict_engine=nc.scalar.copy)

    nc.compile()
    return nc


def _rope_perm_cols():
    """Column permutation of the 256-wide W slice for one core's 4 heads.

    Chunk c (0,1) holds local heads 2c, 2c+1 as rows
    [hA_even(32) | hA_odd(32) | hB_even(32) | hB_odd(32)].
    """
    cols = []
    for c in range(2):
        for j2 in range(2):          # which head within the chunk
            head = 2 * c + j2
            for blk in range(2):     # 0: even dims, 1: odd dims
                for i in range(32):
                    cols.append(head * 64 + 2 * i + blk)
    return np.array(cols)


def _cos_sin_tables():
    inv_freq = 1.0 / (ROPE_THETA ** (np.arange(0, HEAD_DIM, 2, dtype=np.float64)
                                     / HEAD_DIM))          # [32]
    ang = np.arange(S, dtype=np.float64)[None, :] * inv_freq[:, None]  # [32, S]
    cos32 = np.cos(ang).astype(np.float32)
    sin32 = np.sin(ang).astype(np.float32)
    cosT = np.tile(cos32, (4, 1))                           # [128, S]
    # sign: +sin at even-dim rows (blocks 0, 2), -sin at odd-dim rows (1, 3)
    sinT = np.concatenate([sin32, -sin32, sin32, -sin32], axis=0)
    return np.ascontiguousarray(cosT), np.ascontiguousarray(sinT)


def _tile_xT(xT):
    # [1024, 2048] -> [4 sblk, 128 part, 8 kc, 512]
    return np.ascontiguousarray(
        xT.reshape(KC, 128, 4, 512).transpose(2, 1, 0, 3))


def _tile_vT(vT):
    # [1024, 2048] -> [16 sc, 128 part, 8 kc, 128]
    return np.ascontiguousarray(
        vT.reshape(KC, 128, MT, 128).transpose(2, 1, 0, 3))


def _tile_w(w):
    # [1024, 256] -> [128, 8, 256]
    return np.ascontiguousarray(w.reshape(KC, 128, 256).transpose(1, 0, 2))


def _tile_mask(maskT_bf16):
    # [2048, 2048] -> [128, 16 m, 2048]
    return np.ascontiguousarray(
        maskT_bf16.reshape(MT, 128, S).transpose(1, 0, 2))


def kernel(q, k, v, mask, Wq, Wk, Wv, Wo, bo):
    global _BUILT
    if _BUILT is None:
        _BUILT = build_bass()
    nc = _BUILT

    BF = ml_dtypes.bfloat16
    q = np.asarray(q, np.float32)
    k = np.asarray(k, np.float32)
    v = np.asarray(v, np.float32)
    Wq = np.asarray(Wq, np.float32)
    Wk = np.asarray(Wk, np.float32)
    Wv = np.asarray(Wv, np.float32)
    Wo = np.asarray(Wo, np.float32)
    bo = np.asarray(bo, np.float32)
    mask = np.asarray(mask)

    cosT, sinT = _cos_sin_tables()
    perm = _rope_perm_cols()
    qTb = [_tile_xT(q[b].T).astype(BF) for b in range(2)]
    kTb = [_tile_xT(k[b].T).astype(BF) for b in range(2)]
    vTb = [_tile_vT(v[b].T).astype(BF) for b in range(2)]
    maskTb = [_tile_mask(mask[b, 0].T.astype(BF)) for b in range(2)]

    in_maps = []
    for c in range(N_CORES):
        b = c // 4
        head_base = (c % 4) * 4
        cols = slice(head_base * 64, head_base * 64 + 256)
        wo2 = Wo[cols, :].reshape(2, 2, 64, DIM).transpose(1, 2, 0, 3)
        in_maps.append({
            "xq": qTb[b], "xk": kTb[b], "xv": vTb[b],
            "wq": _tile_w(Wq[:, cols][:, perm]).astype(BF),
            "wk": _tile_w(Wk[:, cols][:, perm]).astype(BF),
            "wv": _tile_w(Wv[:, cols]).astype(BF),
            "wo": np.ascontiguousarray(wo2.reshape(128, 2, DIM)).astype(BF),
            "cosT": cosT, "sinT": sinT,
            "maskT": maskTb[b],
        })

    kernel._last_in_maps = in_maps
    res = run_bass_kernel_spmd(nc, in_maps, core_ids=list(range(N_CORES)))
    out = np.zeros((2, S, DIM), np.float32)
    for c in range(N_CORES):
        out[c // 4] += res.results[c]["out_part"]
    out += bo[None, None, :]
    return out


# revision 25
# speedup vs baseline: 1.1220x; 1.1220x over previous
"""Multi-head attention (RoPE + mask + softmax) Trainium2 Bass kernel, v3.

Sharding: 8 cores = 2 batches x 4 head-groups. Core c handles batch c//4,
local heads 4*(c%4) .. +4 (tensor-parallel on heads; Wq/Wk/Wv column-sharded,
Wo row-sharded; per-core partial outputs summed on host).

v3 (from v2 trace: 327us; 82us of PE stalls at interleave boundaries from
rope-swap DMA latency, outproj psum-slot starvation, tail DRAM-bounce):
  - RoPE pair-swap done by DVE stream_shuffle (intra-quadrant, 16-row halves
    via a host-side weight permutation) instead of 4 serialized SBUF DMAs
  - k projected before q; attention iters emitted one sblk behind the
    projections; BOTH units of q-block 0 run inside the projection phase
  - softmax denominator: reciprocal in place on the [1,512] den rows,
    2-row DRAM bounce only for the partition-broadcast read-back
  - outproj spread one token-chunk per 2 m-iters; its psum comes from the
    1-bank avp pool (ps pool bufs=2, avp pool bufs=4 -> 8 banks exactly)
  - deeper at-tile pool (6) so ACT's exp never waits on DVE slot recycling
  - every 4th mask-multiply offloaded to gpsimd in the steady-state units
"""
import sys
sys.path.insert(0, '/opt/trn_rl_repo')
import math
import numpy as np
import ml_dtypes

import concourse.bass as bass
import concourse.mybir as mybir
import concourse.tile as tile
from concourse import bacc
from concourse.bass_utils import run_bass_kernel_spmd

F32 = mybir.dt.float32
BF16 = mybir.dt.bfloat16

S = 2048
DIM = 1024
HEAD_DIM = 64
N_CORES = 8
KC = DIM // 128          # 8 contraction chunks for projections
MT = S // 128            # 16 k-chunks in attention
QB = 512                 # q-block width
NQB = S // QB            # 4
ROPE_THETA = 10000.0
SHUF_MASK = [i ^ 16 for i in range(32)]

_BUILT = None


def build_bass():
    nc = bacc.Bacc("TRN2", target_bir_lowering=False, debug=False)

    xq = nc.dram_tensor("xq", [4, 128, KC, 512], BF16, kind="ExternalInput").ap()
    xk = nc.dram_tensor("xk", [4, 128, KC, 512], BF16, kind="ExternalInput").ap()
    xv = nc.dram_tensor("xv", [MT, 128, KC, 128], BF16, kind="ExternalInput").ap()
    wq = nc.dram_tensor("wq", [128, KC, 256], BF16, kind="ExternalInput").ap()
    wk = nc.dram_tensor("wk", [128, KC, 256], BF16, kind="ExternalInput").ap()
    wv = nc.dram_tensor("wv", [128, KC, 256], BF16, kind="ExternalInput").ap()
    wo = nc.dram_tensor("wo", [128, 2, DIM], BF16, kind="ExternalInput").ap()
    cosT = nc.dram_tensor("cosT", [128, S], F32, kind="ExternalInput").ap()
    sinT = nc.dram_tensor("sinT", [128, S], F32, kind="ExternalInput").ap()
    maskT = nc.dram_tensor("maskT", [128, MT, S], BF16, kind="ExternalInput").ap()
    out_part = nc.dram_tensor("out_part", [S, DIM], F32, kind="ExternalOutput").ap()

    with tile.TileContext(nc) as tc:
        with tc.tile_pool(name="persist", bufs=1) as persist, \
             tc.tile_pool(name="dram", bufs=1, space="DRAM") as dram, \
             tc.tile_pool(name="ps", bufs=3, space="PSUM") as ps, \
             tc.tile_pool(name="avpp", bufs=2, space="PSUM") as avpp, \
             tc.tile_pool(name="xts", bufs=2) as xts, \
             tc.tile_pool(name="rope", bufs=1) as rope, \
             tc.tile_pool(name="atp", bufs=6) as atp, \
             tc.tile_pool(name="atmp", bufs=4) as atmp, \
             tc.tile_pool(name="tailp", bufs=2) as tailp:

            qhT = persist.tile([128, 2, S], BF16)     # [64*h2-chunk, hp, s]
            khT = persist.tile([128, 2, S], BF16)
            vh = persist.tile([128, MT, 4 * 65], BF16)
            outTb = persist.tile([128, 2, S], BF16)   # [pair-packed hd, hp, s]
            wo_sb = persist.tile([128, 2, DIM], BF16)
            cos_sb = persist.tile([128, S], F32)
            sin_sb = persist.tile([128, S], F32)
            mk = persist.tile([128, MT, S], BF16)
            wq_sb = persist.tile([128, KC, 256], BF16)
            wk_sb = persist.tile([128, KC, 256], BF16)
            wv_sb = persist.tile([128, KC, 256], BF16)
            dscr = dram.tile([16, QB], F32)
            dscr2 = dram.tile([16, QB], F32)

            nc.sync.dma_start(out=wk_sb, in_=wk)
            nc.sync.dma_start(out=cos_sb, in_=cosT)
            nc.sync.dma_start(out=sin_sb, in_=sinT)
            nc.sync.dma_start(out=wq_sb, in_=wq)
            nc.sync.dma_start(out=wv_sb, in_=wv)
            nc.sync.dma_start(out=wo_sb, in_=wo)
            # ones column per head for the softmax denominator rows of vh
            nc.vector.memset(
                vh.rearrange("p m (h x) -> p m h x", x=65)[:, :, :, 64:65], 1.0)

            # ---------- emission helpers ----------

            def emit_qk_sblk(xdram, w_sb, dstT, sblk):
                """Project one 512-token block of q or k and apply RoPE."""
                x_sb = xts.tile([128, KC, 512], BF16, tag="xts")
                nc.sync.dma_start(out=x_sb, in_=xdram[sblk])
                ss = slice(sblk * 512, (sblk + 1) * 512)
                psum = ps.tile([128, 2, 512], F32, tag="ps")
                for m in range(2):
                    for kc in range(KC):
                        nc.tensor.matmul(
                            psum[:, m, :],
                            lhsT=w_sb[:, kc, m * 128:(m + 1) * 128],
                            rhs=x_sb[:, kc, :],
                            start=(kc == 0), stop=(kc == KC - 1))
                t = rope.tile([128, 2, 512], F32, tag="t")
                u = rope.tile([128, 2, 512], F32, tag="u")
                us = rope.tile([128, 2, 512], F32, tag="us")
                cosb = cos_sb[:, ss].unsqueeze(1).broadcast_to([128, 2, 512])
                sinb = sin_sb[:, ss].unsqueeze(1).broadcast_to([128, 2, 512])
                nc.vector.tensor_mul(t, psum, cosb)
                nc.vector.tensor_mul(u, psum, sinb)
                nc.vector.stream_shuffle(us, u, SHUF_MASK)
                nc.vector.tensor_add(dstT[:, :, ss], t, us)

            def emit_v_sc(sc):
                """Project one 128-token chunk of v into vh (65-stride)."""
                v_sb = xts.tile([128, KC, 128], BF16, tag="vts")
                nc.sync.dma_start(out=v_sb, in_=xv[sc])
                psum = ps.tile([128, 2, 512], F32, tag="ps")
                for kc in range(KC):
                    nc.tensor.matmul(
                        psum[:, 0, 0:256], lhsT=v_sb[:, kc, :],
                        rhs=wv_sb[:, kc, :],
                        start=(kc == 0), stop=(kc == KC - 1))
                nc.scalar.copy(
                    vh[:, sc, :].rearrange("p (h x) -> p h x", x=65)[:, :, 0:64],
                    psum[:, 0, 0:256].rearrange("p (h x) -> p h x", x=64))

            def emit_mask_dma(m):
                nc.sync.dma_start(out=mk[:, m, :], in_=maskT[:, m, :])

            avps = {}

            def emit_attn_iter(qb, hp, m, mask_eng=None):
                qs = slice(qb * QB, (qb + 1) * QB)
                if m == 0:
                    if (qb, hp) == (0, 1):
                        # phase-A overlapped unit: borrow a ps-pool tile so
                        # avpp can stay 2 tiles for the steady state
                        t = ps.tile([128, 2, 512], F32, tag="ps", name="avp01")
                        avps[(0, 1)] = [t[:, 0, :], t[:, 1, :]]
                    else:
                        avps[(qb, hp)] = [
                            avpp.tile([128, QB], F32, tag="avp",
                                      name=f"avp{qb}{hp}{i}")
                            for i in range(2)]
                avp = avps[(qb, hp)]
                sps = ps.tile([128, 2, 512], F32, tag="ps")
                for h2 in range(2):
                    hb = slice(h2 * 64, (h2 + 1) * 64)
                    nc.tensor.matmul(
                        sps[:, h2, :],
                        lhsT=khT[hb, hp, m * 128:(m + 1) * 128],
                        rhs=qhT[hb, hp, qs],
                        start=True, stop=True)
                at = atp.tile([128, 2, 512], BF16, tag="at")
                nc.scalar.activation(at, sps, mybir.ActivationFunctionType.Exp,
                                     scale=1.0 / math.sqrt(HEAD_DIM))
                atm = atmp.tile([128, 2, 512], BF16, tag="atm")
                mkb = mk[:, m, qs].unsqueeze(1).broadcast_to([128, 2, 512])
                nc.vector.tensor_mul(atm, at, mkb)
                for h2 in range(2):
                    h = 2 * hp + h2
                    nc.tensor.matmul(
                        avp[h2][0:65, :],
                        lhsT=vh[:, m, h * 65:(h + 1) * 65],
                        rhs=atm[:, h2, :],
                        start=(m == 0), stop=(m == MT - 1))

            tails = {}

            def emit_tail_evict(qb, hp):
                """Evict avp psum via DVE copies (frees the avp banks)."""
                avp = avps.pop((qb, hp))
                outF = tailp.tile([128, QB], F32, tag="outF")
                tmpB = tailp.tile([128, QB], F32, tag="tmpB")
                nc.vector.tensor_copy(outF[0:65, :], avp[0][0:65, :])
                nc.vector.tensor_copy(tmpB[0:65, :], avp[1][0:65, :])
                tails[(qb, hp)] = (outF, tmpB)

            def emit_tail_recip(qb, hp):
                """Reciprocal of the den rows via the [128,2,4] DRAM-transpose
                bounce (v1-proven), then shift head B dims into outF."""
                u0 = (qb * 2 + hp) * 2
                outF, tmpB = tails[(qb, hp)]
                nc.sync.dma_start(out=dscr[u0:u0 + 1, :], in_=outF[64:65, :])
                nc.sync.dma_start(out=dscr[u0 + 1:u0 + 2, :], in_=tmpB[64:65, :])
                rin = tailp.tile([128, 2, 4], F32, tag="rin")
                nc.sync.dma_start(
                    out=rin,
                    in_=dscr[u0:u0 + 2].rearrange("u (p f) -> p u f", p=128))
                r32 = tailp.tile([128, 2, 4], F32, tag="r32")
                scr = tailp.tile([128, 2, 4], F32, tag="scr")
                nc.vector.reciprocal_approx_accurate(r32, rin, scr)
                rr = tailp.tile([128, 2, 4], F32, tag="rr")
                nc.vector.tensor_copy(rr, r32)
                nc.sync.dma_start(
                    out=dscr2[u0:u0 + 2].rearrange("u (p f) -> p u f", p=128),
                    in_=rr)
                # head B dims into outF[64:128] (after its den DMA read it)
                nc.sync.dma_start(out=outF[64:128, :], in_=tmpB[0:64, :])

            def emit_tail_norm(qb, hp):
                u0 = (qb * 2 + hp) * 2
                qs = slice(qb * QB, (qb + 1) * QB)
                outF, _ = tails.pop((qb, hp))
                sb_r = tailp.tile([128, QB], F32, tag="sbr")
                nc.sync.dma_start(out=sb_r[0:64, :],
                                  in_=dscr2[u0:u0 + 1].broadcast_to([64, QB]))
                nc.sync.dma_start(out=sb_r[64:128, :],
                                  in_=dscr2[u0 + 1:u0 + 2].broadcast_to([64, QB]))
                nc.vector.tensor_mul(outTb[:, hp, qs], outF, sb_r)

            def emit_outproj_sc(sc, use_scalar=False):
                wps = ps.tile([128, 2, 512], F32, tag="ps")
                tok = slice(sc * 128, (sc + 1) * 128)
                for nb in range(2):
                    for hp in range(2):
                        nc.tensor.matmul(
                            wps[:, nb, :],
                            lhsT=outTb[:, hp, tok],
                            rhs=wo_sb[:, hp, nb * 512:(nb + 1) * 512],
                            start=(hp == 0), stop=(hp == 1))
                co = tailp.tile([128, DIM], F32, tag="co")
                if use_scalar:
                    nc.scalar.copy(co, wps.rearrange("p a b -> p (a b)"))
                else:
                    nc.vector.tensor_copy(co, wps.rearrange("p a b -> p (a b)"))
                nc.sync.dma_start(out=out_part[tok, :], in_=co)

            # ---------- emission schedule ----------

            emit_qk_sblk(xk, wk_sb, khT, 0)
            emit_qk_sblk(xq, wq_sb, qhT, 0)
            for sc in range(4):
                emit_v_sc(sc)
                emit_mask_dma(sc)

            # projection segments with the two qb=0 units pipelined behind
            for s in range(1, 4):
                emit_qk_sblk(xk, wk_sb, khT, s)
                emit_qk_sblk(xq, wq_sb, qhT, s)
                for i in range(4):
                    sc = s * 4 + i
                    emit_v_sc(sc)
                    emit_mask_dma(sc)
                for m in range(4 * (s - 1), 4 * s):
                    emit_attn_iter(0, 0, m)
                if s >= 2:
                    for m in range(4 * (s - 2), 4 * (s - 1)):
                        emit_attn_iter(0, 1, m)
            for m in range(12, 16):
                emit_attn_iter(0, 0, m)
            for m in range(8, 12):
                emit_attn_iter(0, 1, m)
            emit_tail_evict(0, 0)
            emit_tail_recip(0, 0)
            for m in range(12, 16):
                emit_attn_iter(0, 1, m)
            emit_tail_evict(0, 1)

            # steady-state units; tails/outproj pipelined into the next unit
            units = [(1, 0), (1, 1), (2, 0), (2, 1), (3, 0), (3, 1)]
            prevs = [(0, 1), (1, 0), (1, 1), (2, 0), (2, 1), (3, 0)]
            for (qb, hp), prev in zip(units, prevs):
                for m in range(MT):
                    emit_attn_iter(qb, hp, m)
                    if m == 1:
                        emit_tail_recip(*prev)
                    if hp == 0:
                        if m == 2:
                            emit_tail_norm(qb - 1, 0)
                        elif m == 6:
                            emit_tail_norm(qb - 1, 1)
                        elif m in (8, 10, 12, 14):
                            emit_outproj_sc((qb - 1) * 4 + (m - 8) // 2)
                    elif m == 5 and qb == 3:
                        emit_tail_norm(3, 0)
                emit_tail_evict(qb, hp)

            emit_tail_recip(3, 1)
            emit_tail_norm(3, 1)
            for sc in range(12, 16):
                emit_outproj_sc(sc, use_scalar=True)

    nc.compile()
    return nc


def _rope_perm_cols():
    """Column permutation of the 256-wide W slice for one core's 4 heads.

    Within each head, psum row r = 32q + j (q = quadrant half, j = 0..31):
    j < 16  -> even element of rotary pair (16q + j)
    j >= 16 -> odd  element of rotary pair (16q + j - 16)
    so the RoPE swap (even<->odd partner) is row j ^ 16: intra-quadrant,
    doable with one DVE stream_shuffle.
    """
    cols = []
    for head in range(4):
        for q in range(2):
            for half in range(2):    # 0: even elements, 1: odd elements
                for i in range(16):
                    cols.append(head * 64 + 2 * (16 * q + i) + half)
    return np.array(cols)


def _cos_sin_tables():
    inv_freq = 1.0 / (ROPE_THETA ** (np.arange(0, HEAD_DIM, 2, dtype=np.float64)
                                     / HEAD_DIM))          # [32]
    ang = np.arange(S, dtype=np.float64)[None, :] * inv_freq[:, None]  # [32, S]
    cos32 = np.cos(ang)
    sin32 = np.sin(ang)
    cosT = np.zeros((128, S), np.float32)
    sinT = np.zeros((128, S), np.float32)
    for r in range(128):
        rr = r % 64
        q, j = rr // 32, rr % 32
        pi = 16 * q + (j % 16)
        cosT[r] = cos32[pi]
        sinT[r] = sin32[pi] * (1.0 if j < 16 else -1.0)
    return cosT, sinT


def _tile_xT(xT):
    # [1024, 2048] -> [4 sblk, 128 part, 8 kc, 512]
    return np.ascontiguousarray(
        xT.reshape(KC, 128, 4, 512).transpose(2, 1, 0, 3))


def _tile_vT(vT):
    # [1024, 2048] -> [16 sc, 128 part, 8 kc, 128]
    return np.ascontiguousarray(
        vT.reshape(KC, 128, MT, 128).transpose(2, 1, 0, 3))


def _tile_w(w):
    # [1024, 256] -> [128, 8, 256]
    return np.ascontiguousarray(w.reshape(KC, 128, 256).transpose(1, 0, 2))


def _tile_mask(maskT_bf16):
    # [2048, 2048] -> [128, 16 m, 2048]
    return np.ascontiguousarray(
        maskT_bf16.reshape(MT, 128, S).transpose(1, 0, 2))


def kernel(q, k, v, mask, Wq, Wk, Wv, Wo, bo):
    global _BUILT
    if _BUILT is None:
        _BUILT = build_bass()
    nc = _BUILT

    BF = ml_dtypes.bfloat16
    q = np.asarray(q, np.float32)
    k = np.asarray(k, np.float32)
    v = np.asarray(v, np.float32)
    Wq = np.asarray(Wq, np.float32)
    Wk = np.asarray(Wk, np.float32)
    Wv = np.asarray(Wv, np.float32)
    Wo = np.asarray(Wo, np.float32)
    bo = np.asarray(bo, np.float32)
    mask = np.asarray(mask)

    cosT, sinT = _cos_sin_tables()
    perm = _rope_perm_cols()
    qTb = [_tile_xT(q[b].T).astype(BF) for b in range(2)]
    kTb = [_tile_xT(k[b].T).astype(BF) for b in range(2)]
    vTb = [_tile_vT(v[b].T).astype(BF) for b in range(2)]
    maskTb = [_tile_mask(mask[b, 0].T.astype(BF)) for b in range(2)]

    in_maps = []
    for c in range(N_CORES):
        b = c // 4
        head_base = (c % 4) * 4
        cols = slice(head_base * 64, head_base * 64 + 256)
        wo2 = Wo[cols, :].reshape(2, 2, 64, DIM).transpose(1, 2, 0, 3)
        in_maps.append({
            "xq": qTb[b], "xk": kTb[b], "xv": vTb[b],
            "wq": _tile_w(Wq[:, cols][:, perm]).astype(BF),
            "wk": _tile_w(Wk[:, cols][:, perm]).astype(BF),
            "wv": _tile_w(Wv[:, cols]).astype(BF),
            "wo": np.ascontiguousarray(wo2.reshape(128, 2, DIM)).astype(BF),
            "cosT": cosT, "sinT": sinT,
            "maskT": maskTb[b],
        })

    kernel._last_in_maps = in_maps
    res = run_bass_kernel_spmd(nc, in_maps, core_ids=list(range(N_CORES)))
    out = np.zeros((2, S, DIM), np.float32)
    for c in range(N_CORES):
        out[c // 4] += res.results[c]["out_part"]
    out += bo[None, None, :]
    return out
